# revision 13
# baseline (speedup 1.0000x reference)
"""Trainium2 Bass kernel for ContinuousIntegratedKoopmanOperator.

reference: odeint(dz/dt = z @ W) sampled at t = DT*[1..T], y0 = x at t[0].
Closed form (time-invariant linear ODE): out[:, j, :] = x @ expm(DT*j*W).

Strategy:
  host: compute Mj = expm(DT*j*W) in float64; ship fp16 x^T, fp16 table
        for j=0..31 only, and fp16 (E^32)^T. out[:, 0] = x exactly, so
        the device only writes j=1..63.
  device (8 cores, batch-sharded 1024 rows each):
        powers j=32..63 are chained on device: block_{b+8} = E^32 @ block_b
        (4 extra 1024-col matmul units drained back into the fp16 M table),
        interleaved at tile boundaries mid-stream so they never delay the
        early output stream. Work order: all 8 tiles' first halves (seed
        blocks j<32), then all second halves (chained blocks) — so the
        HBM write stream starts as early as possible and never starves.
        Main GEMM: out_half = x_tile @ M_block, single fp16 matmuls, f32
        PSUM, uniform 1024-col units (2 banks), 4-deep PSUM rotation;
        drains alternate Vector/Scalar f32->f16 into 6 rotating half-tile
        staging buffers; outs are per-pair quarters for the very first
        half, then ~1MB halves.
  sync: raw bass, explicit sems; every wait proves a specific event.
"""
import numpy as np

DT = 0.01
B, D, T = 8192, 128, 64
NCORES = 8
BSH = B // NCORES          # 1024 rows per core
NTILES = BSH // 128        # 8 batch tiles per core
BW = 512                   # j-block width (4 j's of 128)
NBLK = (T * D) // BW       # 16 blocks per tile
NSTG = 6                   # half-tile staging buffers
HWID = 8 * BW              # half-tile width (4096 cols)
OW = (T - 1) * D           # 8064 output cols per row (j=1..63)
H0 = HWID - D              # first-half output width (j=1..31): 3968

_CACHE = {}


def _expm_table(W: np.ndarray) -> np.ndarray:
    """(D, T*D) float64: columns [j*D:(j+1)*D] = expm(DT*j*W)."""
    A = DT * W.astype(np.float64)
    M1 = np.eye(D, dtype=np.float64)
    term = np.eye(D, dtype=np.float64)
    for n in range(1, 24):
        term = term @ A / n
        M1 += term
    Ms = np.empty((T, D, D), dtype=np.float64)
    Ms[0] = np.eye(D)
    for j in range(1, T):
        Ms[j] = Ms[j - 1] @ M1
    return np.ascontiguousarray(Ms.transpose(1, 0, 2).reshape(D, T * D))


def _build_nc():
    import concourse.bass as bass
    import concourse.mybir as mybir

    f16 = mybir.dt.float16

    nc = bass.Bass(trn_type="TRN2")
    xT_d = nc.dram_tensor("xT", (D, NTILES * 128), f16, kind="ExternalInput")
    M_d = nc.dram_tensor("M", (D, 8 * BW), f16, kind="ExternalInput")
    E32T_d = nc.dram_tensor("E32T", (D, D), f16, kind="ExternalInput")
    out_d = nc.dram_tensor("out", (BSH, OW), f16, kind="ExternalOutput")

    xT_s = nc.alloc_sbuf_tensor("xT_s", [D, NTILES * 128], f16)
    M_s = nc.alloc_sbuf_tensor("M_s", [D, NBLK * BW], f16)
    E32T_s = nc.alloc_sbuf_tensor("E32T_s", [D, D], f16)
    stg = [nc.alloc_sbuf_tensor(f"stg{p}", [128, HWID], f16) for p in range(NSTG)]
    psum = nc.alloc_psum_tensor("acc", [128, 8 * 512], mybir.dt.float32)

    s_ldx0 = nc.alloc_semaphore("s_ldx0")
    s_ldxr = nc.alloc_semaphore("s_ldxr")
    s_lde = nc.alloc_semaphore("s_lde")
    s_ldm = [nc.alloc_semaphore(f"s_ldm{k}") for k in range(5)]
    s_mm = nc.alloc_semaphore("s_mm")
    s_dv = nc.alloc_semaphore("s_dv")      # Vector drains (even units)
    s_da = nc.alloc_semaphore("s_da")      # Scalar drains (odd units)
    s_osy = [nc.alloc_semaphore(f"s_osy{p}") for p in range(NSTG)]
    s_boot = nc.alloc_semaphore("s_boot")

    all_sems = [s_ldx0, s_ldxr, s_lde, *s_ldm, s_mm, s_dv, s_da, *s_osy, s_boot]
    nums = sorted(s.num for s in all_sems)
    assert nums == list(range(nums[0], nums[-1] + 1)), "sems not contiguous"
    sem_range = range(nums[0], nums[-1] + 1)

    nc.gpsimd.dma_reset(sem_range)

    # M chunk per seed block (load gating): b0, b1, b2-3, b4-5, b6-7
    LDM_OF_BLOCK = {0: 0, 1: 1, 2: 2, 3: 2, 4: 3, 5: 3, 6: 4, 7: 4}

    # --- the unit stream ---
    # ("m", i, q): 2 main matmuls x_tile_i @ blocks (2q, 2q+1)
    # ("c", t):    chain unit, M blocks 8+2t,9+2t = E^32 @ blocks 2t,2t+1
    # halves: half k<8 = (tile k, pairs 0-3); half k>=8 = (tile k-8, 4-7).
    # Chain units are interposed at tile boundaries after the first two
    # first-halves so they never gate the early output stream.
    units = []
    half_units = {}                     # half index -> its 4 unit positions
    k = 0
    for i in range(NTILES):
        if 2 <= i <= 5:
            units.append(("c", i - 2))
        half_units[k] = []
        for q in range(4):
            half_units[k].append(len(units))
            units.append(("m", i, q))
        k += 1
    for i in range(NTILES):
        half_units[k] = []
        for q in range(4, 8):
            half_units[k].append(len(units))
            units.append(("m", i, q))
        k += 1
    NHALF = k                           # 16

    def dr_sem(U):
        return s_dv if U % 2 == 0 else s_da

    def dr_val(U):
        return U // 2 + 1

    def stage_of(k):
        return k % NSTG

    # out-DMA count for half k (very first half goes out as 4 quarters)
    def outs_of(k):
        return 4 if k == 0 else 1

    def outs_before(p, n):
        return sum(outs_of(k) for k in range(n) if stage_of(k) == p)

    # per-half out waits: each drain engine's max ordinal over the half's units
    def half_waits(k):
        us = half_units[k]
        vv = max((dr_val(U) for U in us if U % 2 == 0), default=0)
        aa = max((dr_val(U) for U in us if U % 2 == 1), default=0)
        return vv, aa

    with nc.Block() as block:
        @block.sync
        def _(sync):
            sync.sem_clear(sem_range)
            sync.nop().then_inc(s_boot, 1)
            # loads, ordered to track PE consumption during ramp
            sync.dma_start(out=xT_s[:, 0:128], in_=xT_d[:, 0:128]).then_inc(s_ldx0, 16)
            sync.dma_start(out=M_s[:, 0:512], in_=M_d[:, 0:512]).then_inc(s_ldm[0], 16)
            sync.dma_start(out=M_s[:, 512:1024], in_=M_d[:, 512:1024]).then_inc(s_ldm[1], 16)
            sync.dma_start(out=M_s[:, 1024:2048], in_=M_d[:, 1024:2048]).then_inc(s_ldm[2], 16)
            sync.dma_start(out=xT_s[:, 128:], in_=xT_d[:, 128:]).then_inc(s_ldxr, 16)
            sync.dma_start(out=M_s[:, 2048:3072], in_=M_d[:, 2048:3072]).then_inc(s_ldm[3], 16)
            sync.dma_start(out=M_s[:, 3072:4096], in_=M_d[:, 3072:4096]).then_inc(s_ldm[4], 16)
            sync.dma_start(out=E32T_s[:, :], in_=E32T_d[:, :]).then_inc(s_lde, 16)
            # outs; j=0 (stg cols 0:128 of half 0 stages) never written
            for k in range(NHALF):
                p = stage_of(k)
                i = k % NTILES
                h = k // NTILES
                if k == 0:
                    for lq in range(4):      # per-pair quarters, earliest bytes
                        U = half_units[0][lq]
                        sync.wait_ge(dr_sem(U), dr_val(U))
                        c0 = max(lq * 1024, D)
                        sync.dma_start(
                            out=out_d[0:128, c0 - D:(lq + 1) * 1024 - D],
                            in_=stg[p][:, c0:(lq + 1) * 1024]).then_inc(s_osy[p], 16)
                    continue
                vv, aa = half_waits(k)
                sync.wait_ge(s_dv, vv)
                sync.wait_ge(s_da, aa)
                if h == 0:
                    sync.dma_start(out=out_d[i * 128:(i + 1) * 128, 0:H0],
                                   in_=stg[p][:, D:HWID]).then_inc(s_osy[p], 16)
                else:
                    sync.dma_start(out=out_d[i * 128:(i + 1) * 128, H0:OW],
                                   in_=stg[p][:, 0:HWID]).then_inc(s_osy[p], 16)
            for p in range(NSTG):
                sync.wait_ge(s_osy[p], 16 * outs_before(p, NHALF))

        @block.tensor
        def _(tensor):
            tensor.wait_ge(s_boot, 1)
            chain_waited = False
            for U, u in enumerate(units):
                if u[0] == "m":
                    _, i, q = u
                    if i == 0 and q == 0:
                        tensor.wait_ge(s_ldx0, 16)
                    if i == 1 and q == 0:
                        tensor.wait_ge(s_ldxr, 16)
                    if q >= 4 and not chain_waited:
                        # blocks 8-15 all chained by now: last chain drains
                        tensor.wait_ge(s_dv, max(
                            dr_val(W) for W, w in enumerate(units)
                            if w[0] == "c" and W % 2 == 0))
                        tensor.wait_ge(s_da, max(
                            dr_val(W) for W, w in enumerate(units)
                            if w[0] == "c" and W % 2 == 1))
                        chain_waited = True
                else:
                    if u[1] == 0:
                        tensor.wait_ge(s_lde, 16)
                if U >= 4:                      # PSUM slot reused: drained?
                    tensor.wait_ge(dr_sem(U - 4), dr_val(U - 4))
                pb = (U % 4) * 1024
                for r in range(2):
                    if u[0] == "m":
                        _, i, q = u
                        b = 2 * q + r
                        if i == 0 and b in LDM_OF_BLOCK:
                            tensor.wait_ge(s_ldm[LDM_OF_BLOCK[b]], 16)
                        lhsT = xT_s[:, i * 128:(i + 1) * 128]
                        rhs = M_s[:, b * BW:(b + 1) * BW]
                    else:
                        t = u[1]
                        b = 2 * t + r
                        tensor.wait_ge(s_ldm[LDM_OF_BLOCK[b]], 16)
                        lhsT = E32T_s[:, :]
                        rhs = M_s[:, b * BW:(b + 1) * BW]
                    tensor.matmul(psum[:, pb + r * 512:pb + (r + 1) * 512],
                                  lhsT, rhs, start=True, stop=True).then_inc(s_mm, 1)

        def drain_stream(eng, parity):
            eng.wait_ge(s_boot, 1)
            # map unit -> (half k, local pair) for main units
            unit_half = {}
            for k2, us in half_units.items():
                for lq, U in enumerate(us):
                    unit_half[U] = (k2, lq)
            seen_halves = set()
            for U, u in enumerate(units):
                if U % 2 != parity:
                    continue
                eng.wait_ge(s_mm, 2 * (U + 1))  # both matmuls of unit U
                pb = (U % 4) * 1024
                sem = s_dv if parity == 0 else s_da
                if u[0] == "m":
                    k2, lq = unit_half[U]
                    p = stage_of(k2)
                    if k2 >= NSTG and k2 not in seen_halves:
                        eng.wait_ge(s_osy[p], 16 * outs_before(p, k2 - NSTG + 1))
                    seen_halves.add(k2)
                    c0 = D if (k2 < NTILES and lq == 0) else lq * 1024
                    dst = stg[p][:, c0:(lq + 1) * 1024]
                    src = psum[:, pb + c0 - lq * 1024:pb + 1024]
                else:
                    t = u[1]
                    dst = M_s[:, (8 + 2 * t) * BW:(10 + 2 * t) * BW]
                    src = psum[:, pb:pb + 1024]
                if parity == 0:
                    eng.tensor_copy(out=dst, in_=src).then_inc(sem, 1)
                else:
                    eng.copy(out=dst, in_=src).then_inc(sem, 1)

        @block.vector
        def _(vector):
            drain_stream(vector, 0)

        @block.scalar
        def _(scalar):
            drain_stream(scalar, 1)

    return nc


def _prep_inputs(x: np.ndarray, Mcat64: np.ndarray):
    """Per-core input maps from the (D, T*D) float64 expm table."""
    Mb = np.ascontiguousarray(Mcat64[:, :32 * D]).astype(np.float16)
    E32T = np.ascontiguousarray(Mcat64[:, 32 * D:33 * D].T).astype(np.float16)
    maps = []
    for c in range(NCORES):
        xc = np.ascontiguousarray(x[c * BSH:(c + 1) * BSH].T.astype(np.float16))
        maps.append({"xT": xc, "M": Mb, "E32T": E32T})
    return maps


def run_on_device(x: np.ndarray, Mcat64: np.ndarray, trace: bool = False):
    from concourse.bass_utils import run_bass_kernel_spmd

    if "nc" not in _CACHE:
        _CACHE["nc"] = _build_nc()
    nc = _CACHE["nc"]

    in_maps = _prep_inputs(x, Mcat64)
    res = run_bass_kernel_spmd(nc, in_maps, core_ids=list(range(NCORES)), trace=trace)
    out = np.empty((B, T, D), dtype=np.float32)
    for c in range(NCORES):
        blk = out[c * BSH:(c + 1) * BSH]
        blk[:, 0, :] = x[c * BSH:(c + 1) * BSH]
        blk[:, 1:, :] = (
            res.results[c]["out"].astype(np.float32).reshape(BSH, T - 1, D))
    return out, res


def kernel(x, W, T):
    x = np.asarray(x, dtype=np.float32)
    W = np.asarray(W, dtype=np.float32)
    assert int(T) == 64 and x.shape == (B, D) and W.shape == (D, D)
    Mcat64 = _expm_table(W)
    out, _ = run_on_device(x, Mcat64, trace=False)
    return out


# revision 14
# speedup vs baseline: 1.0682x; 1.0682x over previous
"""Trainium2 Bass kernel for ContinuousIntegratedKoopmanOperator.

reference: odeint(dz/dt = z @ W) sampled at t = DT*[1..T], y0 = x at t[0].
Closed form (time-invariant linear ODE): out[:, j, :] = x @ expm(DT*j*W).

Strategy:
  host: compute Mj = expm(DT*j*W) in float64; ship fp16 x^T, fp16 table
        for j=0..31 only, and fp16 (E^32)^T. out[:, 0] = x exactly, so
        the device only writes j=1..63.
  device (8 cores, batch-sharded 1024 rows each):
        powers j=32..63 are chained on device: block_{b+8} = E^32 @ block_b
        (4 extra 1024-col matmul units, drained back into the fp16 M table).
        Main GEMM: out_tile = x @ M_block, single fp16 matmuls, f32 PSUM.
        Uniform 1024-col units (2 PSUM banks), 4-deep PSUM rotation;
        drains alternate Vector/Scalar f32->f16 into staging; outs are
        per-pair quarters for tile 0 (stream starts ASAP) then ~1MB halves.
        Warmup matmuls while loads land keep the PE p-state ramping.
  sync: raw bass, explicit sems; every wait proves a specific event.
"""
import numpy as np

DT = 0.01
B, D, T = 8192, 128, 64
NCORES = 8
BSH = B // NCORES          # 1024 rows per core
NTILES = BSH // 128        # 8 batch tiles per core
BW = 512                   # j-block width (4 j's of 128)
NBLK = (T * D) // BW       # 16 blocks per tile
NPAIR = 8                  # block-pairs per tile (unit = 2 banks)
NSTG = 3                   # staging buffers
OW = (T - 1) * D           # 8064 output cols per row (j=1..63)
H0 = 4096 - D              # half-0 width: stg cols [128,4096) -> out [0,3968)
NWARM = 10                 # PE warmup matmuls (512 cols each)

_CACHE = {}


def _expm_table(W: np.ndarray) -> np.ndarray:
    """(D, T*D) float64: columns [j*D:(j+1)*D] = expm(DT*j*W)."""
    A = DT * W.astype(np.float64)
    M1 = np.eye(D, dtype=np.float64)
    term = np.eye(D, dtype=np.float64)
    for n in range(1, 24):
        term = term @ A / n
        M1 += term
    Ms = np.empty((T, D, D), dtype=np.float64)
    Ms[0] = np.eye(D)
    for j in range(1, T):
        Ms[j] = Ms[j - 1] @ M1
    return np.ascontiguousarray(Ms.transpose(1, 0, 2).reshape(D, T * D))


def _build_nc():
    import concourse.bass as bass
    import concourse.mybir as mybir

    f16 = mybir.dt.float16

    nc = bass.Bass(trn_type="TRN2")
    xT_d = nc.dram_tensor("xT", (D, NTILES * 128), f16, kind="ExternalInput")
    M_d = nc.dram_tensor("M", (D, 8 * BW), f16, kind="ExternalInput")
    E32T_d = nc.dram_tensor("E32T", (D, D), f16, kind="ExternalInput")
    out_d = nc.dram_tensor("out", (BSH, OW), f16, kind="ExternalOutput")

    xT_s = nc.alloc_sbuf_tensor("xT_s", [D, NTILES * 128], f16)
    M_s = nc.alloc_sbuf_tensor("M_s", [D, NBLK * BW], f16)
    E32T_s = nc.alloc_sbuf_tensor("E32T_s", [D, D], f16)
    stg = [nc.alloc_sbuf_tensor(f"stg{p}", [128, NBLK * BW], f16) for p in range(NSTG)]
    psum = nc.alloc_psum_tensor("acc", [128, 8 * 512], mybir.dt.float32)

    s_ldx0 = nc.alloc_semaphore("s_ldx0")
    s_ldxr = nc.alloc_semaphore("s_ldxr")
    s_lde = nc.alloc_semaphore("s_lde")
    s_ldm = [nc.alloc_semaphore(f"s_ldm{k}") for k in range(4)]
    s_mm = nc.alloc_semaphore("s_mm")
    s_dv = nc.alloc_semaphore("s_dv")      # Vector drains (even units)
    s_da = nc.alloc_semaphore("s_da")      # Scalar drains (odd units)
    s_osy = [nc.alloc_semaphore(f"s_osy{p}") for p in range(NSTG)]
    s_boot = nc.alloc_semaphore("s_boot")

    all_sems = [s_ldx0, s_ldxr, s_lde, *s_ldm, s_mm, s_dv, s_da, *s_osy, s_boot]
    nums = sorted(s.num for s in all_sems)
    assert nums == list(range(nums[0], nums[-1] + 1)), "sems not contiguous"
    sem_range = range(nums[0], nums[-1] + 1)

    nc.gpsimd.dma_reset(sem_range)

    # --- the unit stream: ("m", tile, pair) = 2 main matmuls (blocks 2q,
    # 2q+1); ("c", t) = chain unit producing M blocks 8+2t, 9+2t from
    # blocks 2t, 2t+1 via stationary (E^32)^T. Unit U -> PSUM slot U%4,
    # drain engine V if U even else S, per-engine ordinal U//2.
    units = []
    for q in range(4):
        units.append(("m", 0, q))
        units.append(("c", q))
    for q in range(4, NPAIR):
        units.append(("m", 0, q))
    for i in range(1, NTILES):
        for q in range(NPAIR):
            units.append(("m", i, q))

    def dr_sem(U):
        return s_dv if U % 2 == 0 else s_da

    def dr_val(U):
        return U // 2 + 1

    U_of_mq = {(i, q): U for U, u in enumerate(units)
               if u[0] == "m" for (_, i, q) in [u]}

    # number of out-DMAs for tiles with index < n mapping to staging p
    # (tile 0 goes out in 5 pieces, later tiles in 2 halves)
    def outs_before(p, n):
        return sum((5 if i == 0 else 2) for i in range(n) if i % NSTG == p)

    with nc.Block() as block:
        @block.sync
        def _(sync):
            sync.sem_clear(sem_range)
            sync.nop().then_inc(s_boot, 1)
            # loads, ordered to track PE consumption during ramp
            sync.dma_start(out=xT_s[:, 0:128], in_=xT_d[:, 0:128]).then_inc(s_ldx0, 16)
            sync.dma_start(out=M_s[:, 0:1024], in_=M_d[:, 0:1024]).then_inc(s_ldm[0], 16)
            sync.dma_start(out=E32T_s[:, :], in_=E32T_d[:, :]).then_inc(s_lde, 16)
            sync.dma_start(out=M_s[:, 1024:2048], in_=M_d[:, 1024:2048]).then_inc(s_ldm[1], 16)
            sync.dma_start(out=M_s[:, 2048:3072], in_=M_d[:, 2048:3072]).then_inc(s_ldm[2], 16)
            sync.dma_start(out=xT_s[:, 128:], in_=xT_d[:, 128:]).then_inc(s_ldxr, 16)
            sync.dma_start(out=M_s[:, 3072:4096], in_=M_d[:, 3072:4096]).then_inc(s_ldm[3], 16)
            # outs; j=0 (stg cols 0:128) never written. Tile 0 in per-pair
            # quarters so the write stream starts ASAP; later tiles in halves.
            for q in range(4):
                U = U_of_mq[(0, q)]
                sync.wait_ge(dr_sem(U), dr_val(U))
                c0 = max(q * 1024, D)
                sync.dma_start(out=out_d[0:128, c0 - D:(q + 1) * 1024 - D],
                               in_=stg[0][:, c0:(q + 1) * 1024]).then_inc(s_osy[0], 16)
            U = U_of_mq[(0, 7)]
            sync.wait_ge(s_dv, dr_val(U - 1))
            sync.wait_ge(s_da, dr_val(U))
            sync.dma_start(out=out_d[0:128, H0:OW],
                           in_=stg[0][:, 4096:8192]).then_inc(s_osy[0], 16)
            for i in range(1, NTILES):
                p = i % NSTG
                for h in range(2):
                    U = U_of_mq[(i, 4 * h + 3)]
                    sync.wait_ge(s_dv, dr_val(U - 1))
                    sync.wait_ge(s_da, dr_val(U))
                    if h == 0:
                        sync.dma_start(out=out_d[i * 128:(i + 1) * 128, 0:H0],
                                       in_=stg[p][:, D:D + H0]).then_inc(s_osy[p], 16)
                    else:
                        sync.dma_start(out=out_d[i * 128:(i + 1) * 128, H0:OW],
                                       in_=stg[p][:, 4096:8192]).then_inc(s_osy[p], 16)
            for p in range(NSTG):
                sync.wait_ge(s_osy[p], 16 * outs_before(p, NTILES))

        @block.tensor
        def _(tensor):
            tensor.wait_ge(s_boot, 1)
            # warmup: ramp the PE p-state while the M table is still
            # loading. Results land in PSUM slot 3 bank 7, which the real
            # stream only reuses at unit 7 (after a drain-gated overwrite).
            tensor.wait_ge(s_ldx0, 16)
            tensor.wait_ge(s_ldm[0], 16)
            for _w in range(NWARM):
                tensor.matmul(psum[:, 7 * 512:8 * 512], xT_s[:, 0:128],
                              M_s[:, 0:BW], start=True, stop=True)
            for U, u in enumerate(units):
                if u[0] == "m":
                    _, i, q = u
                    if i == 0:
                        if q == 0:
                            tensor.wait_ge(s_ldx0, 16)
                            tensor.wait_ge(s_ldm[0], 16)
                        elif q < 4:
                            tensor.wait_ge(s_ldm[q], 16)
                        else:
                            # blocks 8+2t,9+2t come from chain unit t's drain
                            tensor.wait_ge(s_da, q - 4 + 1)
                    if i == 1 and q == 0:
                        tensor.wait_ge(s_ldxr, 16)
                else:
                    t = u[1]
                    if t == 0:
                        tensor.wait_ge(s_lde, 16)
                    else:
                        tensor.wait_ge(s_ldm[t], 16)
                if U >= 4:                      # PSUM slot reused: drained?
                    tensor.wait_ge(dr_sem(U - 4), dr_val(U - 4))
                pb = (U % 4) * 1024
                for r in range(2):
                    if u[0] == "m":
                        _, i, q = u
                        lhsT = xT_s[:, i * 128:(i + 1) * 128]
                        rhs = M_s[:, (2 * q + r) * BW:(2 * q + r + 1) * BW]
                    else:
                        t = u[1]
                        lhsT = E32T_s[:, :]
                        rhs = M_s[:, (2 * t + r) * BW:(2 * t + r + 1) * BW]
                    tensor.matmul(psum[:, pb + r * 512:pb + (r + 1) * 512],
                                  lhsT, rhs, start=True, stop=True).then_inc(s_mm, 1)

        def drain_stream(eng, parity):
            eng.wait_ge(s_boot, 1)
            seen_tiles = set()
            for U, u in enumerate(units):
                if U % 2 != parity:
                    continue
                eng.wait_ge(s_mm, 2 * (U + 1))  # both matmuls of unit U
                pb = (U % 4) * 1024
                sem = s_dv if parity == 0 else s_da
                if u[0] == "m":
                    _, i, q = u
                    p = i % NSTG
                    if i >= NSTG and i not in seen_tiles:
                        eng.wait_ge(s_osy[p], 16 * outs_before(p, i - NSTG + 1))
                    seen_tiles.add(i)
                    c0 = D if q == 0 else q * 1024  # j=0 cols never drained
                    dst = stg[p][:, c0:(q + 1) * 1024]
                    src = psum[:, pb + c0 - q * 1024:pb + 1024]
                else:
                    t = u[1]
                    dst = M_s[:, (8 + 2 * t) * BW:(10 + 2 * t) * BW]
                    src = psum[:, pb:pb + 1024]
                if parity == 0:
                    eng.tensor_copy(out=dst, in_=src).then_inc(sem, 1)
                else:
                    eng.copy(out=dst, in_=src).then_inc(sem, 1)

        @block.vector
        def _(vector):
            drain_stream(vector, 0)

        @block.scalar
        def _(scalar):
            drain_stream(scalar, 1)

    return nc


def _prep_inputs(x: np.ndarray, Mcat64: np.ndarray):
    """Per-core input maps from the (D, T*D) float64 expm table."""
    Mb = np.ascontiguousarray(Mcat64[:, :32 * D]).astype(np.float16)
    E32T = np.ascontiguousarray(Mcat64[:, 32 * D:33 * D].T).astype(np.float16)
    maps = []
    for c in range(NCORES):
        xc = np.ascontiguousarray(x[c * BSH:(c + 1) * BSH].T.astype(np.float16))
        maps.append({"xT": xc, "M": Mb, "E32T": E32T})
    return maps


def run_on_device(x: np.ndarray, Mcat64: np.ndarray, trace: bool = False):
    from concourse.bass_utils import run_bass_kernel_spmd

    if "nc" not in _CACHE:
        _CACHE["nc"] = _build_nc()
    nc = _CACHE["nc"]

    in_maps = _prep_inputs(x, Mcat64)
    res = run_bass_kernel_spmd(nc, in_maps, core_ids=list(range(NCORES)), trace=trace)
    out = np.empty((B, T, D), dtype=np.float32)
    for c in range(NCORES):
        blk = out[c * BSH:(c + 1) * BSH]
        blk[:, 0, :] = x[c * BSH:(c + 1) * BSH]
        blk[:, 1:, :] = (
            res.results[c]["out"].astype(np.float32).reshape(BSH, T - 1, D))
    return out, res


def kernel(x, W, T):
    x = np.asarray(x, dtype=np.float32)
    W = np.asarray(W, dtype=np.float32)
    assert int(T) == 64 and x.shape == (B, D) and W.shape == (D, D)
    Mcat64 = _expm_table(W)
    out, _ = run_on_device(x, Mcat64, trace=False)
    return out


# revision 20
# speedup vs baseline: 1.1778x; 1.1026x over previous
"""Trainium2 Bass kernel for ContinuousIntegratedKoopmanOperator.

reference: odeint(dz/dt = z @ W) sampled at t = DT*[1..T], y0 = x at t[0].
Closed form (time-invariant linear ODE): out[:, j, :] = x @ expm(DT*j*W).

Strategy:
  host: compute Mj = expm(DT*j*W) in float64; ship fp16 x^T, fp16 table
        for j=0..31 only, and fp16 (E^32)^T. out[:, 0] = x exactly, so
        the device only writes j=1..63.
  device (8 cores, batch-sharded 1024 rows each):
        powers j=32..63 are chained on device: block_{b+8} = E^32 @ block_b
        (4 extra 1024-col matmul units, drained back into the fp16 M table).
        Main GEMM: out_tile = x @ M_block, single fp16 matmuls, f32 PSUM.
        Uniform 1024-col units (2 PSUM banks), 4-deep PSUM rotation;
        drains alternate Vector/Scalar f32->f16 into staging; outs are
        per-pair quarters for tile 0 (stream starts ASAP) then ~1MB halves.
        Warmup matmuls while loads land keep the PE p-state ramping.
  sync: raw bass, explicit sems; every wait proves a specific event.
"""
import numpy as np

DT = 0.01
B, D, T = 8192, 128, 64
NCORES = 8
BSH = B // NCORES          # 1024 rows per core
NTILES = BSH // 128        # 8 batch tiles per core
BW = 512                   # j-block width (4 j's of 128)
NBLK = (T * D) // BW       # 16 blocks per tile
NPAIR = 8                  # block-pairs per tile (unit = 2 banks)
NSTG = 3                   # staging buffers
OW = (T - 1) * D           # 8064 output cols per row (j=1..63)
H0 = 4096 - D              # half-0 width: stg cols [128,4096) -> out [0,3968)
NWARM = 8                  # PE warmup matmuls (128 cols each)

_CACHE = {}


def _expm_table(W: np.ndarray) -> np.ndarray:
    """(D, T*D) float64: columns [j*D:(j+1)*D] = expm(DT*j*W)."""
    A = DT * W.astype(np.float64)
    M1 = np.eye(D, dtype=np.float64)
    term = np.eye(D, dtype=np.float64)
    for n in range(1, 24):
        term = term @ A / n
        M1 += term
    Ms = np.empty((T, D, D), dtype=np.float64)
    Ms[0] = np.eye(D)
    for j in range(1, T):
        Ms[j] = Ms[j - 1] @ M1
    return np.ascontiguousarray(Ms.transpose(1, 0, 2).reshape(D, T * D))


def _build_nc():
    import concourse.bass as bass
    import concourse.mybir as mybir

    f16 = mybir.dt.float16

    nc = bass.Bass(trn_type="TRN2")
    xT_d = nc.dram_tensor("xT", (D, NTILES * 128), f16, kind="ExternalInput")
    M_d = nc.dram_tensor("M", (D, 8 * BW), f16, kind="ExternalInput")
    E32T_d = nc.dram_tensor("E32T", (D, D), f16, kind="ExternalInput")
    out_d = nc.dram_tensor("out", (BSH, OW), f16, kind="ExternalOutput")

    xT_s = nc.alloc_sbuf_tensor("xT_s", [D, NTILES * 128], f16)
    M_s = nc.alloc_sbuf_tensor("M_s", [D, NBLK * BW], f16)
    E32T_s = nc.alloc_sbuf_tensor("E32T_s", [D, D], f16)
    stg = [nc.alloc_sbuf_tensor(f"stg{p}", [128, NBLK * BW], f16) for p in range(NSTG)]
    psum = nc.alloc_psum_tensor("acc", [128, 8 * 512], mybir.dt.float32)

    s_ldx0 = nc.alloc_semaphore("s_ldx0")
    s_ldxr = nc.alloc_semaphore("s_ldxr")
    s_lde = nc.alloc_semaphore("s_lde")
    s_ldm = [nc.alloc_semaphore(f"s_ldm{k}") for k in range(4)]
    s_mm = nc.alloc_semaphore("s_mm")
    s_dv = nc.alloc_semaphore("s_dv")      # Vector drains (even units)
    s_da = nc.alloc_semaphore("s_da")      # Scalar drains (odd units)
    s_osy = [nc.alloc_semaphore(f"s_osy{p}") for p in range(NSTG)]
    s_boot = nc.alloc_semaphore("s_boot")

    all_sems = [s_ldx0, s_ldxr, s_lde, *s_ldm, s_mm, s_dv, s_da, *s_osy, s_boot]
    nums = sorted(s.num for s in all_sems)
    assert nums == list(range(nums[0], nums[-1] + 1)), "sems not contiguous"
    sem_range = range(nums[0], nums[-1] + 1)

    nc.gpsimd.dma_reset(sem_range)

    # --- the unit stream: ("m", tile, pair) = 2 main matmuls (blocks 2q,
    # 2q+1); ("c", t) = chain unit producing M blocks 8+2t, 9+2t from
    # blocks 2t, 2t+1 via stationary (E^32)^T. Unit U -> PSUM slot U%4,
    # drain engine V if U even else S, per-engine ordinal U//2.
    units = []
    for q in range(4):
        units.append(("m", 0, q))
    for t in range(4):
        units.append(("c", t))
    for q in range(4, NPAIR):
        units.append(("m", 0, q))
    for i in range(1, NTILES):
        for q in range(NPAIR):
            units.append(("m", i, q))
    U_of_c = [U for U, u in enumerate(units) if u[0] == "c"]

    def dr_sem(U):
        return s_dv if U % 2 == 0 else s_da

    def dr_val(U):
        return U // 2 + 1

    U_of_mq = {(i, q): U for U, u in enumerate(units)
               if u[0] == "m" for (_, i, q) in [u]}

    # number of out-DMAs for tiles with index < n mapping to staging p
    # (tile 0 goes out in 5 pieces, later tiles in 2 halves)
    def outs_before(p, n):
        return sum((5 if i == 0 else 2) for i in range(n) if i % NSTG == p)

    with nc.Block() as block:
        @block.sync
        def _(sync):
            sync.sem_clear(sem_range)
            sync.nop().then_inc(s_boot, 1)
            # loads, ordered to track PE consumption during ramp
            sync.dma_start(out=xT_s[:, 0:128], in_=xT_d[:, 0:128]).then_inc(s_ldx0, 16)
            sync.dma_start(out=M_s[:, 0:1024], in_=M_d[:, 0:1024]).then_inc(s_ldm[0], 16)
            sync.dma_start(out=M_s[:, 1024:2048], in_=M_d[:, 1024:2048]).then_inc(s_ldm[1], 16)
            sync.dma_start(out=E32T_s[:, :], in_=E32T_d[:, :]).then_inc(s_lde, 16)
            sync.dma_start(out=M_s[:, 2048:3072], in_=M_d[:, 2048:3072]).then_inc(s_ldm[2], 16)
            sync.dma_start(out=xT_s[:, 128:], in_=xT_d[:, 128:]).then_inc(s_ldxr, 16)
            sync.dma_start(out=M_s[:, 3072:4096], in_=M_d[:, 3072:4096]).then_inc(s_ldm[3], 16)
            # outs; j=0 (stg cols 0:128) never written. Tile 0 in per-pair
            # quarters so the write stream starts ASAP; later tiles in halves.
            for q in range(4):
                U = U_of_mq[(0, q)]
                sync.wait_ge(dr_sem(U), dr_val(U))
                c0 = max(q * 1024, D)
                sync.dma_start(out=out_d[0:128, c0 - D:(q + 1) * 1024 - D],
                               in_=stg[0][:, c0:(q + 1) * 1024]).then_inc(s_osy[0], 16)
            U = U_of_mq[(0, 7)]
            sync.wait_ge(s_dv, dr_val(U - 1))
            sync.wait_ge(s_da, dr_val(U))
            sync.dma_start(out=out_d[0:128, H0:OW],
                           in_=stg[0][:, 4096:8192]).then_inc(s_osy[0], 16)
            for i in range(1, NTILES):
                p = i % NSTG
                for h in range(2):
                    U = U_of_mq[(i, 4 * h + 3)]
                    sync.wait_ge(s_dv, dr_val(U - 1))
                    sync.wait_ge(s_da, dr_val(U))
                    if h == 0:
                        sync.dma_start(out=out_d[i * 128:(i + 1) * 128, 0:H0],
                                       in_=stg[p][:, D:D + H0]).then_inc(s_osy[p], 16)
                    else:
                        sync.dma_start(out=out_d[i * 128:(i + 1) * 128, H0:OW],
                                       in_=stg[p][:, 4096:8192]).then_inc(s_osy[p], 16)
            for p in range(NSTG):
                sync.wait_ge(s_osy[p], 16 * outs_before(p, NTILES))

        @block.tensor
        def _(tensor):
            tensor.wait_ge(s_boot, 1)
            # warmup: ramp the PE p-state while the M table is still
            # loading — x0 alone suffices (results are never read). They
            # land in PSUM slot 3 bank 7, which the real stream only
            # reuses at unit 3 (same-engine ordering, overwritten).
            tensor.wait_ge(s_ldx0, 16)
            for _w in range(NWARM):
                tensor.matmul(psum[:, 7 * 512:7 * 512 + 128], xT_s[:, 0:128],
                              xT_s[:, 0:128], start=True, stop=True)
            for U, u in enumerate(units):
                if u[0] == "m":
                    _, i, q = u
                    if i == 0:
                        if q == 0:
                            tensor.wait_ge(s_ldx0, 16)
                            tensor.wait_ge(s_ldm[0], 16)
                        elif q < 4:
                            tensor.wait_ge(s_ldm[q], 16)
                        else:
                            # blocks 8+2t,9+2t come from chain unit t's drain
                            Uc = U_of_c[q - 4]
                            tensor.wait_ge(dr_sem(Uc), dr_val(Uc))
                    if i == 1 and q == 0:
                        tensor.wait_ge(s_ldxr, 16)
                else:
                    t = u[1]
                    if t == 0:
                        tensor.wait_ge(s_lde, 16)
                    else:
                        tensor.wait_ge(s_ldm[t], 16)
                if U >= 4:                      # PSUM slot reused: drained?
                    tensor.wait_ge(dr_sem(U - 4), dr_val(U - 4))
                pb = (U % 4) * 1024
                for r in range(2):
                    if u[0] == "m":
                        _, i, q = u
                        lhsT = xT_s[:, i * 128:(i + 1) * 128]
                        rhs = M_s[:, (2 * q + r) * BW:(2 * q + r + 1) * BW]
                    else:
                        t = u[1]
                        lhsT = E32T_s[:, :]
                        rhs = M_s[:, (2 * t + r) * BW:(2 * t + r + 1) * BW]
                    tensor.matmul(psum[:, pb + r * 512:pb + (r + 1) * 512],
                                  lhsT, rhs, start=True, stop=True).then_inc(s_mm, 1)

        def drain_stream(eng, parity):
            eng.wait_ge(s_boot, 1)
            seen_tiles = set()
            for U, u in enumerate(units):
                if U % 2 != parity:
                    continue
                eng.wait_ge(s_mm, 2 * (U + 1))  # both matmuls of unit U
                pb = (U % 4) * 1024
                sem = s_dv if parity == 0 else s_da
                if u[0] == "m":
                    _, i, q = u
                    p = i % NSTG
                    if i >= NSTG and i not in seen_tiles:
                        eng.wait_ge(s_osy[p], 16 * outs_before(p, i - NSTG + 1))
                    seen_tiles.add(i)
                    c0 = D if q == 0 else q * 1024  # j=0 cols never drained
                    dst = stg[p][:, c0:(q + 1) * 1024]
                    src = psum[:, pb + c0 - q * 1024:pb + 1024]
                else:
                    t = u[1]
                    dst = M_s[:, (8 + 2 * t) * BW:(10 + 2 * t) * BW]
                    src = psum[:, pb:pb + 1024]
                if parity == 0:
                    eng.tensor_copy(out=dst, in_=src).then_inc(sem, 1)
                else:
                    eng.copy(out=dst, in_=src).then_inc(sem, 1)

        @block.vector
        def _(vector):
            drain_stream(vector, 0)

        @block.scalar
        def _(scalar):
            drain_stream(scalar, 1)

    return nc


def _prep_inputs(x: np.ndarray, Mcat64: np.ndarray):
    """Per-core input maps from the (D, T*D) float64 expm table."""
    Mb = np.ascontiguousarray(Mcat64[:, :32 * D]).astype(np.float16)
    E32T = np.ascontiguousarray(Mcat64[:, 32 * D:33 * D].T).astype(np.float16)
    maps = []
    for c in range(NCORES):
        xc = np.ascontiguousarray(x[c * BSH:(c + 1) * BSH].T.astype(np.float16))
        maps.append({"xT": xc, "M": Mb, "E32T": E32T})
    return maps


def run_on_device(x: np.ndarray, Mcat64: np.ndarray, trace: bool = False):
    from concourse.bass_utils import run_bass_kernel_spmd

    if "nc" not in _CACHE:
        _CACHE["nc"] = _build_nc()
    nc = _CACHE["nc"]

    in_maps = _prep_inputs(x, Mcat64)
    res = run_bass_kernel_spmd(nc, in_maps, core_ids=list(range(NCORES)), trace=trace)
    out = np.empty((B, T, D), dtype=np.float32)
    for c in range(NCORES):
        blk = out[c * BSH:(c + 1) * BSH]
        blk[:, 0, :] = x[c * BSH:(c + 1) * BSH]
        blk[:, 1:, :] = (
            res.results[c]["out"].astype(np.float32).reshape(BSH, T - 1, D))
    return out, res


def kernel(x, W, T):
    x = np.asarray(x, dtype=np.float32)
    W = np.asarray(W, dtype=np.float32)
    assert int(T) == 64 and x.shape == (B, D) and W.shape == (D, D)
    Mcat64 = _expm_table(W)
    out, _ = run_on_device(x, Mcat64, trace=False)
    return out


# revision 22
# speedup vs baseline: 1.1820x; 1.0035x over previous
"""Trainium2 Bass kernel for ContinuousIntegratedKoopmanOperator.

reference: odeint(dz/dt = z @ W) sampled at t = DT*[1..T], y0 = x at t[0].
Closed form (time-invariant linear ODE): out[:, j, :] = x @ expm(DT*j*W).

Strategy:
  host: compute Mj = expm(DT*j*W) in float64; ship fp16 x^T, fp16 table
        for j=0..31 only, and fp16 (E^32)^T. out[:, 0] = x exactly, so
        the device only writes j=1..63.
  device (8 cores, batch-sharded 1024 rows each):
        powers j=32..63 are chained on device: block_{b+8} = E^32 @ block_b
        (4 extra 1024-col matmul units, drained back into the fp16 M table).
        Main GEMM: out_tile = x @ M_block, single fp16 matmuls, f32 PSUM.
        Uniform 1024-col units (2 PSUM banks), 4-deep PSUM rotation;
        drains alternate Vector/Scalar f32->f16 into staging; outs are
        per-pair quarters for tile 0 (stream starts ASAP) then ~1MB halves.
        Warmup matmuls while loads land keep the PE p-state ramping.
  sync: raw bass, explicit sems; every wait proves a specific event.
"""
import numpy as np

DT = 0.01
B, D, T = 8192, 128, 64
NCORES = 8
BSH = B // NCORES          # 1024 rows per core
NTILES = BSH // 128        # 8 batch tiles per core
BW = 512                   # j-block width (4 j's of 128)
NBLK = (T * D) // BW       # 16 blocks per tile
NPAIR = 8                  # block-pairs per tile (unit = 2 banks)
NSTG = 3                   # staging buffers
OW = (T - 1) * D           # 8064 output cols per row (j=1..63)
H0 = 4096 - D              # half-0 width: stg cols [128,4096) -> out [0,3968)
NWARM = 8                  # PE warmup matmuls (128 cols each)

_CACHE = {}


def _expm_table(W: np.ndarray) -> np.ndarray:
    """(D, T*D) float64: columns [j*D:(j+1)*D] = expm(DT*j*W)."""
    A = DT * W.astype(np.float64)
    M1 = np.eye(D, dtype=np.float64)
    term = np.eye(D, dtype=np.float64)
    for n in range(1, 24):
        term = term @ A / n
        M1 += term
    Ms = np.empty((T, D, D), dtype=np.float64)
    Ms[0] = np.eye(D)
    for j in range(1, T):
        Ms[j] = Ms[j - 1] @ M1
    return np.ascontiguousarray(Ms.transpose(1, 0, 2).reshape(D, T * D))


def _build_nc():
    import concourse.bass as bass
    import concourse.mybir as mybir

    f16 = mybir.dt.float16

    nc = bass.Bass(trn_type="TRN2")
    xT_d = nc.dram_tensor("xT", (D, NTILES * 128), f16, kind="ExternalInput")
    M_d = nc.dram_tensor("M", (D, 8 * BW), f16, kind="ExternalInput")
    E32T_d = nc.dram_tensor("E32T", (D, D), f16, kind="ExternalInput")
    out_d = nc.dram_tensor("out", (BSH, OW), f16, kind="ExternalOutput")

    xT_s = nc.alloc_sbuf_tensor("xT_s", [D, NTILES * 128], f16)
    M_s = nc.alloc_sbuf_tensor("M_s", [D, NBLK * BW], f16)
    E32T_s = nc.alloc_sbuf_tensor("E32T_s", [D, D], f16)
    stg = [nc.alloc_sbuf_tensor(f"stg{p}", [128, NBLK * BW], f16) for p in range(NSTG)]
    psum = nc.alloc_psum_tensor("acc", [128, 8 * 512], mybir.dt.float32)

    s_ldx0 = nc.alloc_semaphore("s_ldx0")
    s_ldxr = nc.alloc_semaphore("s_ldxr")
    s_lde = nc.alloc_semaphore("s_lde")
    s_ldm = [nc.alloc_semaphore(f"s_ldm{k}") for k in range(4)]
    s_mm = nc.alloc_semaphore("s_mm")
    s_dv = nc.alloc_semaphore("s_dv")      # Vector drains (even units)
    s_da = nc.alloc_semaphore("s_da")      # Scalar drains (odd units)
    s_osy = [nc.alloc_semaphore(f"s_osy{p}") for p in range(NSTG)]
    s_boot = nc.alloc_semaphore("s_boot")

    all_sems = [s_ldx0, s_ldxr, s_lde, *s_ldm, s_mm, s_dv, s_da, *s_osy, s_boot]
    nums = sorted(s.num for s in all_sems)
    assert nums == list(range(nums[0], nums[-1] + 1)), "sems not contiguous"
    sem_range = range(nums[0], nums[-1] + 1)

    nc.gpsimd.dma_reset(sem_range)

    # --- the unit stream: ("m", tile, pair) = 2 main matmuls (blocks 2q,
    # 2q+1); ("c", t) = chain unit producing M blocks 8+2t, 9+2t from
    # blocks 2t, 2t+1 via stationary (E^32)^T. Unit U -> PSUM slot U%4,
    # drain engine V if U even else S, per-engine ordinal U//2.
    units = []
    for q in range(4):
        units.append(("m", 0, q))
    for t in range(4):
        units.append(("c", t))
    for q in range(4, NPAIR):
        units.append(("m", 0, q))
    for i in range(1, NTILES):
        for q in range(NPAIR):
            units.append(("m", i, q))
    U_of_c = [U for U, u in enumerate(units) if u[0] == "c"]

    def dr_sem(U):
        return s_dv if U % 2 == 0 else s_da

    def dr_val(U):
        return U // 2 + 1

    U_of_mq = {(i, q): U for U, u in enumerate(units)
               if u[0] == "m" for (_, i, q) in [u]}

    # number of out-DMAs for tiles with index < n mapping to staging p
    # (tiles 0-2 go out in 8 per-pair pieces, later tiles in 2 halves)
    def outs_before(p, n):
        return sum((8 if i < 3 else 2) for i in range(n) if i % NSTG == p)

    with nc.Block() as block:
        @block.sync
        def _(sync):
            sync.sem_clear(sem_range)
            sync.nop().then_inc(s_boot, 1)
            # loads, ordered to track PE consumption during ramp
            sync.dma_start(out=xT_s[:, 0:128], in_=xT_d[:, 0:128]).then_inc(s_ldx0, 16)
            sync.dma_start(out=M_s[:, 0:1024], in_=M_d[:, 0:1024]).then_inc(s_ldm[0], 16)
            sync.dma_start(out=M_s[:, 1024:2048], in_=M_d[:, 1024:2048]).then_inc(s_ldm[1], 16)
            sync.dma_start(out=E32T_s[:, :], in_=E32T_d[:, :]).then_inc(s_lde, 16)
            sync.dma_start(out=M_s[:, 2048:3072], in_=M_d[:, 2048:3072]).then_inc(s_ldm[2], 16)
            sync.dma_start(out=xT_s[:, 128:], in_=xT_d[:, 128:]).then_inc(s_ldxr, 16)
            sync.dma_start(out=M_s[:, 3072:4096], in_=M_d[:, 3072:4096]).then_inc(s_ldm[3], 16)
            # outs; j=0 (stg cols 0:128) never written. Tiles 0-2 go out per
            # pair (256KB) so the write stream never stalls during the ramp;
            # tiles 3-7 in ~1MB halves.
            for i in range(3):
                p = i % NSTG
                for q in range(NPAIR):
                    U = U_of_mq[(i, q)]
                    sync.wait_ge(dr_sem(U), dr_val(U))
                    c0 = max(q * 1024, D)
                    sync.dma_start(
                        out=out_d[i * 128:(i + 1) * 128,
                                  c0 - D:(q + 1) * 1024 - D],
                        in_=stg[p][:, c0:(q + 1) * 1024]).then_inc(s_osy[p], 16)
            for i in range(3, NTILES):
                p = i % NSTG
                for h in range(2):
                    U = U_of_mq[(i, 4 * h + 3)]
                    sync.wait_ge(s_dv, dr_val(U - 1))
                    sync.wait_ge(s_da, dr_val(U))
                    if h == 0:
                        sync.dma_start(out=out_d[i * 128:(i + 1) * 128, 0:H0],
                                       in_=stg[p][:, D:D + H0]).then_inc(s_osy[p], 16)
                    else:
                        sync.dma_start(out=out_d[i * 128:(i + 1) * 128, H0:OW],
                                       in_=stg[p][:, 4096:8192]).then_inc(s_osy[p], 16)
            for p in range(NSTG):
                sync.wait_ge(s_osy[p], 16 * outs_before(p, NTILES))

        @block.tensor
        def _(tensor):
            tensor.wait_ge(s_boot, 1)
            # warmup: ramp the PE p-state while the M table is still
            # loading — x0 alone suffices (results are never read). They
            # land in PSUM slot 3 bank 7, which the real stream only
            # reuses at unit 3 (same-engine ordering, overwritten).
            tensor.wait_ge(s_ldx0, 16)
            for _w in range(NWARM):
                tensor.matmul(psum[:, 7 * 512:7 * 512 + 128], xT_s[:, 0:128],
                              xT_s[:, 0:128], start=True, stop=True)
            for U, u in enumerate(units):
                if u[0] == "m":
                    _, i, q = u
                    if i == 0:
                        if q == 0:
                            tensor.wait_ge(s_ldx0, 16)
                            tensor.wait_ge(s_ldm[0], 16)
                        elif q < 4:
                            tensor.wait_ge(s_ldm[q], 16)
                        else:
                            # blocks 8+2t,9+2t come from chain unit t's drain
                            Uc = U_of_c[q - 4]
                            tensor.wait_ge(dr_sem(Uc), dr_val(Uc))
                    if i == 1 and q == 0:
                        tensor.wait_ge(s_ldxr, 16)
                else:
                    t = u[1]
                    if t == 0:
                        tensor.wait_ge(s_lde, 16)
                    else:
                        tensor.wait_ge(s_ldm[t], 16)
                if U >= 4:                      # PSUM slot reused: drained?
                    tensor.wait_ge(dr_sem(U - 4), dr_val(U - 4))
                pb = (U % 4) * 1024
                for r in range(2):
                    if u[0] == "m":
                        _, i, q = u
                        lhsT = xT_s[:, i * 128:(i + 1) * 128]
                        rhs = M_s[:, (2 * q + r) * BW:(2 * q + r + 1) * BW]
                    else:
                        t = u[1]
                        lhsT = E32T_s[:, :]
                        rhs = M_s[:, (2 * t + r) * BW:(2 * t + r + 1) * BW]
                    tensor.matmul(psum[:, pb + r * 512:pb + (r + 1) * 512],
                                  lhsT, rhs, start=True, stop=True).then_inc(s_mm, 1)

        def drain_stream(eng, parity):
            eng.wait_ge(s_boot, 1)
            seen_tiles = set()
            for U, u in enumerate(units):
                if U % 2 != parity:
                    continue
                eng.wait_ge(s_mm, 2 * (U + 1))  # both matmuls of unit U
                pb = (U % 4) * 1024
                sem = s_dv if parity == 0 else s_da
                if u[0] == "m":
                    _, i, q = u
                    p = i % NSTG
                    if i >= NSTG and i not in seen_tiles:
                        eng.wait_ge(s_osy[p], 16 * outs_before(p, i - NSTG + 1))
                    seen_tiles.add(i)
                    c0 = D if q == 0 else q * 1024  # j=0 cols never drained
                    dst = stg[p][:, c0:(q + 1) * 1024]
                    src = psum[:, pb + c0 - q * 1024:pb + 1024]
                else:
                    t = u[1]
                    dst = M_s[:, (8 + 2 * t) * BW:(10 + 2 * t) * BW]
                    src = psum[:, pb:pb + 1024]
                if parity == 0:
                    eng.tensor_copy(out=dst, in_=src).then_inc(sem, 1)
                else:
                    eng.copy(out=dst, in_=src).then_inc(sem, 1)

        @block.vector
        def _(vector):
            drain_stream(vector, 0)

        @block.scalar
        def _(scalar):
            drain_stream(scalar, 1)

    return nc


def _prep_inputs(x: np.ndarray, Mcat64: np.ndarray):
    """Per-core input maps from the (D, T*D) float64 expm table."""
    Mb = np.ascontiguousarray(Mcat64[:, :32 * D]).astype(np.float16)
    E32T = np.ascontiguousarray(Mcat64[:, 32 * D:33 * D].T).astype(np.float16)
    maps = []
    for c in range(NCORES):
        xc = np.ascontiguousarray(x[c * BSH:(c + 1) * BSH].T.astype(np.float16))
        maps.append({"xT": xc, "M": Mb, "E32T": E32T})
    return maps


def run_on_device(x: np.ndarray, Mcat64: np.ndarray, trace: bool = False):
    from concourse.bass_utils import run_bass_kernel_spmd

    if "nc" not in _CACHE:
        _CACHE["nc"] = _build_nc()
    nc = _CACHE["nc"]

    in_maps = _prep_inputs(x, Mcat64)
    res = run_bass_kernel_spmd(nc, in_maps, core_ids=list(range(NCORES)), trace=trace)
    out = np.empty((B, T, D), dtype=np.float32)
    for c in range(NCORES):
        blk = out[c * BSH:(c + 1) * BSH]
        blk[:, 0, :] = x[c * BSH:(c + 1) * BSH]
        blk[:, 1:, :] = (
            res.results[c]["out"].astype(np.float32).reshape(BSH, T - 1, D))
    return out, res


def kernel(x, W, T):
    x = np.asarray(x, dtype=np.float32)
    W = np.asarray(W, dtype=np.float32)
    assert int(T) == 64 and x.shape == (B, D) and W.shape == (D, D)
    Mcat64 = _expm_table(W)
    out, _ = run_on_device(x, Mcat64, trace=False)
    return out
